# revision 31
# baseline (speedup 1.0000x reference)
"""Trainium2 Bass kernel for nn_Attentional_Aggregation (segment softmax attention).

Math (reference):
    keys_i = emb_i @ Wk.T + bk
    q_g    = emb[last(g)] @ Wq.T + bq
    logit_i = <q_{g(i)}, keys_i>
    w = segment_softmax(logit)
    out_g = sum_{i in g} w_i * keys_i

Reformulation (same as baseline):
    logit_i = <qk_{g(i)}, emb_i> + cq_{g(i)},  qk_g = embL_g @ (Wq.T Wk) + bq Wk
    The additive cq term cancels in the softmax and is never computed.
    out_g = (sum e_i emb_i / sum e_i) @ Wk.T + bk   (division + Wk projection on host)

Device strategy v2 (per core; instruction-count-minimal):
  Host packs WHOLE groups into 128-element tiles with <=16 group slots per
  tile (avg group ~10 elems).  32 tiles = 1 superblock (SB).  Per SB:
    - DMA embT    [128c, 32*128i]  (host-transposed, bf16)
    - DMA embt1   [128i, 32*129]   (natural + ones column per tile, bf16)
    - DMA mask    [128i, 32*16]    (host one-hot of element->group-slot, bf16)
    - 32 logits MMs: L[:, 16t:16t+16] = embT_t.T @ qk_win_t   (one PSUM bank)
    - 1 ACT exp [128, 512] PSUM->SBUF bf16
    - 1 DVE  me = mask * E          [128, 512] bf16
    - 32 scatter MMs (4x col-tiled): Nm[32j:32j+16, 129k:+129] = me_t.T @ embt1_t
      -> numerator cols 0..127, denominator col 128, 8 tiles per PSUM bank
    - 4 DVE copies [128, 258] -> bf16 SBUF, 4 DMAs out
  qk table ([128c, 16*NT] bf16) is computed on host and stays SBUF-resident.
  Host: out_g = (num_g / den_g) @ Wk.T + bk  (vectorized sgemm).
"""

import os
import numpy as np
import ml_dtypes

import concourse.bacc as bacc
import concourse.bass as bass
import concourse.mybir as mybir
import concourse.tile as tile
from concourse.bass_utils import run_bass_kernel_spmd

BF16 = ml_dtypes.bfloat16

N = 1_000_000
G = 100_000
D = 128
NCORES = 8
W = 16            # group slots per tile
TPB = 32          # tiles per superblock
ROWS = 129        # 128 emb cols + ones column

# Exposed for test harness
LAST_EXEC_NS = None
LAST_RESULTS = None

_cache = {}


def _build_program(NSB, ncores=NCORES):
    f32 = mybir.dt.float32
    bf16 = mybir.dt.bfloat16

    nc = bacc.Bacc(
        "TRN2",
        target_bir_lowering=False,
        debug=False,
        enable_asserts=False,
        num_devices=ncores,
    )

    f8 = mybir.dt.float8e4
    NT = NSB * TPB
    embt1 = nc.dram_tensor("embt1", [NSB, 128, TPB * ROWS], bf16, kind="ExternalInput").ap()
    embTt = nc.dram_tensor("embTt", [NSB, 128, TPB * 128], bf16, kind="ExternalInput").ap()
    maskh = nc.dram_tensor("maskh", [NSB, 128, TPB * W], f8, kind="ExternalInput").ap()
    qkth = nc.dram_tensor("qkth", [128, NT * W], bf16, kind="ExternalInput").ap()
    outp = nc.dram_tensor("outp", [NSB, 128, 8 * ROWS], bf16, kind="ExternalOutput").ap()

    with tile.TileContext(nc) as tc:
        with (
            tc.tile_pool(name="cpool", bufs=1) as cpool,
            tc.tile_pool(name="pemb", bufs=5) as pemb,
            tc.tile_pool(name="pembT", bufs=5) as pembT,
            tc.tile_pool(name="pmask", bufs=5) as pmask,
            tc.tile_pool(name="plog", bufs=2, space="PSUM") as plog,
            tc.tile_pool(name="pe", bufs=2) as pe_,
            tc.tile_pool(name="pme", bufs=2) as pme,
            tc.tile_pool(name="pnm", bufs=4, space="PSUM") as pnm,
            tc.tile_pool(name="posb", bufs=4) as posb,
        ):
            qk_sb = cpool.tile([128, NT * W], bf16)
            # load the qk table in per-SB chunks (keeps individual DMAs modest)
            for s in range(NSB):
                nc.gpsimd.dma_start(
                    out=qk_sb[:, s * TPB * W : (s + 1) * TPB * W],
                    in_=qkth[:, s * TPB * W : (s + 1) * TPB * W],
                )

            for sb in range(NSB):
                # spread the big loads across the three DMA-capable queues
                # (sync / scalar / gpsimd), ~26MB each
                et = pemb.tile([128, TPB * ROWS], bf16)
                nc.sync.dma_start(out=et[:], in_=embt1[sb])
                eT = pembT.tile([128, TPB * 128], bf16)
                nc.scalar.dma_start(out=eT[:], in_=embTt[sb])
                mk = pmask.tile([128, TPB * W], f8)
                nc.gpsimd.dma_start(out=mk[:], in_=maskh[sb])

                # all-pairs logits for each tile against its 16 group slots
                L = plog.tile([128, TPB * W], f32, space="PSUM")
                for t in range(TPB):
                    nc.tensor.matmul(
                        L[:, W * t : W * (t + 1)],
                        lhsT=eT[:, 128 * t : 128 * (t + 1)],
                        rhs=qk_sb[:, W * (TPB * sb + t) : W * (TPB * sb + t + 1)],
                        start=True,
                        stop=True,
                    )
                E = pe_.tile([128, TPB * W], bf16)
                nc.scalar.activation(E[:], L[:], mybir.ActivationFunctionType.Exp)
                me = pme.tile([128, TPB * W], bf16)
                nc.vector.tensor_mul(me[:], mk[:], E[:])

                # scatter: numerator + denominator per group slot
                ob = posb.tile([128, 8 * ROWS], bf16)
                for q in range(4):
                    sp = pnm.tile([128, 512], f32, space="PSUM")
                    for r in range(8):
                        t = 8 * q + r
                        j = r % 4
                        k = r // 4
                        nc.tensor.matmul(
                            sp[32 * j : 32 * j + 16, ROWS * k : ROWS * (k + 1)],
                            lhsT=me[:, W * t : W * (t + 1)],
                            rhs=et[:, ROWS * t : ROWS * (t + 1)],
                            start=True,
                            stop=True,
                            tile_position=(0, 32 * j),
                        )
                    nc.vector.tensor_copy(
                        ob[:, 2 * ROWS * q : 2 * ROWS * (q + 1)], sp[:, : 2 * ROWS]
                    )
                nc.gpsimd.dma_start(out=outp[sb], in_=ob[:])

    nc.compile()
    return nc


def _pack_core(counts):
    """Best-fit-decreasing pack of whole groups into tiles (<=128 elems,
    <=W groups per tile).

    Returns tile_of_group, relg_of_group (slot within tile), elem_offset_of_group
    (start position of the group's elements within its tile), n_tiles.
    """
    ng = len(counts)
    order = np.argsort(-counts, kind="stable")
    tile_of_group = np.empty(ng, dtype=np.int64)
    relg_of_group = np.empty(ng, dtype=np.int64)
    off_of_group = np.empty(ng, dtype=np.int64)
    # buckets[c] = list of tile ids with remaining capacity exactly c (and
    # an open group slot)
    buckets = [[] for _ in range(129)]
    cap = []
    slots = []
    for g in order:
        c = int(counts[g])
        # best fit: smallest remaining capacity >= c
        for r in range(c, 129):
            if buckets[r]:
                t = buckets[r].pop()
                break
        else:
            t = len(cap)
            cap.append(128)
            slots.append(0)
        tile_of_group[g] = t
        relg_of_group[g] = slots[t]
        off_of_group[g] = 128 - cap[t]
        cap[t] -= c
        slots[t] += 1
        if slots[t] < W and cap[t] > 0:
            buckets[cap[t]].append(t)
    return tile_of_group, relg_of_group, off_of_group, len(cap)


def _host_prep(embeddings, seg_ids, Wq, bq, Wk, bk, ncores=NCORES, num_groups=G):
    emb = np.ascontiguousarray(embeddings, dtype=np.float32)
    seg = np.ascontiguousarray(seg_ids, dtype=np.int64)
    n = len(seg)

    counts = np.bincount(seg, minlength=num_groups)
    assert counts.max() <= 128, "group larger than one tile"
    cum = np.cumsum(counts)
    starts = cum - counts

    ARm = (Wq.T @ Wk).astype(np.float32)
    uvec = (bq @ Wk).astype(np.float32)

    # split groups across cores at ~equal element counts
    bounds = [0]
    for c in range(1, ncores):
        gidx = int(np.searchsorted(cum, n * c // ncores))
        bounds.append(min(max(gidx, bounds[-1]), num_groups))
    bounds.append(num_groups)

    packs = []
    NT_max = 0
    for c in range(ncores):
        g0, g1 = bounds[c], bounds[c + 1]
        tog, rog, oog, ntile = _pack_core(counts[g0:g1])
        packs.append((g0, g1, tog, rog, oog))
        NT_max = max(NT_max, ntile)
    NSB = (NT_max + TPB - 1) // TPB
    NT = NSB * TPB

    emb_bf = emb.astype(BF16)
    in_maps = []
    decs = []
    for c in range(ncores):
        g0, g1, tog, rog, oog = packs[c]
        e0, e1 = int(starts[g0]), int(cum[g1 - 1])
        segc = seg[e0:e1] - g0
        # element placement
        T_e = tog[segc]                       # tile of each element
        pos_e = (np.arange(e0, e1) - starts[seg[e0:e1]]) + oog[segc]
        sb_e = T_e // TPB
        t_e = T_e % TPB

        nat = np.zeros((NSB, TPB, 128, 128), dtype=BF16)   # [sb, t, i, c]
        nat[sb_e, t_e, pos_e] = emb_bf[e0:e1]

        embt1 = np.empty((NSB, 128, TPB, ROWS), dtype=BF16)
        embt1[:, :, :, 128] = BF16(1.0)
        embt1[:, :, :, :128] = nat.transpose(0, 2, 1, 3)
        embt1 = embt1.reshape(NSB, 128, TPB * ROWS)

        embTt = np.ascontiguousarray(
            nat.transpose(0, 3, 1, 2).reshape(NSB, 128, TPB * 128)
        )
        del nat

        mask = np.zeros((NSB, 128, TPB, W), dtype=ml_dtypes.float8_e4m3)
        mask[sb_e, pos_e, t_e, rog[segc]] = 1.0
        mask = mask.reshape(NSB, 128, TPB * W)

        # host qk table: qk_g = embL_g @ ARm + u
        last_idx = cum[g0:g1] - 1
        qk = emb[last_idx] @ ARm + uvec                     # [ng, 128] f32
        qkT = np.zeros((128, NT * W), dtype=BF16)
        gslot = tog * W + rog                               # [ng]
        qkT[:, gslot] = qk.T.astype(BF16)

        in_maps.append(
            dict(
                embt1=np.ascontiguousarray(embt1),
                embTt=embTt,
                maskh=np.ascontiguousarray(mask),
                qkth=np.ascontiguousarray(qkT),
            )
        )
        # decode indices: group -> outp[sb, 32*j + relg, 258*q + 129*k + c]
        t_in_sb = tog % TPB
        q_g = t_in_sb // 8
        rem = t_in_sb % 8
        row_g = 32 * (rem % 4) + rog
        col_g = 2 * ROWS * q_g + ROWS * (rem // 4)
        decs.append((g0, g1, tog // TPB, row_g, col_g))
    return NSB, in_maps, decs


def kernel(embeddings, seg_ids, Wq, bq, Wk, bk):
    global LAST_EXEC_NS, LAST_RESULTS
    Wq = np.asarray(Wq, dtype=np.float32)
    bq = np.asarray(bq, dtype=np.float32)
    Wk = np.asarray(Wk, dtype=np.float32)
    bk = np.asarray(bk, dtype=np.float32)
    embeddings = np.asarray(embeddings)
    seg_ids = np.asarray(seg_ids)

    NSB, in_maps, decs = _host_prep(embeddings, seg_ids, Wq, bq, Wk, bk)

    if NSB not in _cache:
        _cache[NSB] = _build_program(NSB)
    nc = _cache[NSB]

    trace = bool(int(os.environ.get("BASS_KERNEL_TRACE", "0")))
    res = run_bass_kernel_spmd(nc, in_maps, core_ids=list(range(NCORES)), trace=trace)
    LAST_RESULTS = res
    LAST_EXEC_NS = res.exec_time_ns

    num = np.empty((G, D), dtype=np.float32)
    den = np.empty((G,), dtype=np.float32)
    for c in range(NCORES):
        g0, g1, sb_g, row_g, col_g = decs[c]
        o = res.results[c]["outp"].astype(np.float32)
        num[g0:g1] = o[sb_g[:, None], row_g[:, None], col_g[:, None] + np.arange(128)]
        den[g0:g1] = o[sb_g, row_g, col_g + 128]
    out = (num / den[:, None]) @ Wk.T + bk
    return out.astype(np.float32)


# revision 36
# speedup vs baseline: 1.0217x; 1.0217x over previous
"""Trainium2 Bass kernel for nn_Attentional_Aggregation (segment softmax attention).

Math (reference):
    keys_i = emb_i @ Wk.T + bk
    q_g    = emb[last(g)] @ Wq.T + bq
    logit_i = <q_{g(i)}, keys_i>
    w = segment_softmax(logit)
    out_g = sum_{i in g} w_i * keys_i

Reformulation (same as baseline):
    logit_i = <qk_{g(i)}, emb_i> + cq_{g(i)},  qk_g = embL_g @ (Wq.T Wk) + bq Wk
    The additive cq term cancels in the softmax and is never computed.
    out_g = (sum e_i emb_i / sum e_i) @ Wk.T + bk   (division + Wk projection on host)

Device strategy v2 (per core; instruction-count-minimal):
  Host packs WHOLE groups into 128-element tiles with <=16 group slots per
  tile (avg group ~10 elems).  32 tiles = 1 superblock (SB).  Per SB:
    - DMA embT    [128c, 32*128i]  (host-transposed, bf16)
    - DMA embt1   [128i, 32*129]   (natural + ones column per tile, bf16)
    - DMA mask    [128i, 32*16]    (host one-hot of element->group-slot, bf16)
    - 32 logits MMs: L[:, 16t:16t+16] = embT_t.T @ qk_win_t   (one PSUM bank)
    - 1 ACT exp [128, 512] PSUM->SBUF bf16
    - 1 DVE  me = mask * E          [128, 512] bf16
    - 32 scatter MMs (4x col-tiled): Nm[32j:32j+16, 129k:+129] = me_t.T @ embt1_t
      -> numerator cols 0..127, denominator col 128, 8 tiles per PSUM bank
    - 4 DVE copies [128, 258] -> bf16 SBUF, 4 DMAs out
  qk table ([128c, 16*NT] bf16) is computed on host and stays SBUF-resident.
  Host: out_g = (num_g / den_g) @ Wk.T + bk  (vectorized sgemm).
"""

import os
import numpy as np
import ml_dtypes

import concourse.bacc as bacc
import concourse.bass as bass
import concourse.mybir as mybir
import concourse.tile as tile
from concourse.bass_utils import run_bass_kernel_spmd

BF16 = ml_dtypes.bfloat16

N = 1_000_000
G = 100_000
D = 128
NCORES = 8
W = 16            # group slots per tile
TPB = 32          # tiles per superblock
ROWS = 129        # 128 emb cols + ones column

# Exposed for test harness
LAST_EXEC_NS = None
LAST_RESULTS = None

_cache = {}


def _build_program(NSB, ncores=NCORES):
    f32 = mybir.dt.float32
    bf16 = mybir.dt.bfloat16

    nc = bacc.Bacc(
        "TRN2",
        target_bir_lowering=False,
        debug=False,
        enable_asserts=False,
        num_devices=ncores,
    )

    f8 = mybir.dt.float8e4
    NT = NSB * TPB
    embt1 = nc.dram_tensor("embt1", [NSB, 128, TPB * ROWS], bf16, kind="ExternalInput").ap()
    embTt = nc.dram_tensor("embTt", [NSB, 128, TPB * 128], bf16, kind="ExternalInput").ap()
    maskh = nc.dram_tensor("maskh", [NSB, 128, TPB * W], f8, kind="ExternalInput").ap()
    qkth = nc.dram_tensor("qkth", [128, NT * W], bf16, kind="ExternalInput").ap()
    outp = nc.dram_tensor("outp", [NSB, 4, 16, 8 * ROWS], bf16, kind="ExternalOutput").ap()

    with tile.TileContext(nc) as tc:
        with (
            tc.tile_pool(name="cpool", bufs=1) as cpool,
            tc.tile_pool(name="pemb", bufs=5) as pemb,
            tc.tile_pool(name="pembT", bufs=5) as pembT,
            tc.tile_pool(name="pmask", bufs=5) as pmask,
            tc.tile_pool(name="plog", bufs=2, space="PSUM") as plog,
            tc.tile_pool(name="pe", bufs=2) as pe_,
            tc.tile_pool(name="pme", bufs=2) as pme,
            tc.tile_pool(name="pnm", bufs=4, space="PSUM") as pnm,
            tc.tile_pool(name="posb", bufs=4) as posb,
        ):
            qk_sb = cpool.tile([128, NT * W], bf16)

            for sb in range(NSB):
                # stream the qk chunk for this SB (plus one ahead) just in time
                for s in ([0, 1] if sb == 0 else [sb + 1]):
                    if s < NSB:
                        nc.gpsimd.dma_start(
                            out=qk_sb[:, s * TPB * W : (s + 1) * TPB * W],
                            in_=qkth[:, s * TPB * W : (s + 1) * TPB * W],
                        )
                # spread the big loads across the three DMA-capable queues
                # (sync / scalar / gpsimd), ~26MB each
                et = pemb.tile([128, TPB * ROWS], bf16)
                nc.sync.dma_start(out=et[:], in_=embt1[sb])
                eT = pembT.tile([128, TPB * 128], bf16)
                nc.scalar.dma_start(out=eT[:], in_=embTt[sb])
                mk = pmask.tile([128, TPB * W], f8)
                nc.gpsimd.dma_start(out=mk[:], in_=maskh[sb])

                # all-pairs logits for each tile against its 16 group slots
                L = plog.tile([128, TPB * W], f32, space="PSUM")
                for t in range(TPB):
                    nc.tensor.matmul(
                        L[:, W * t : W * (t + 1)],
                        lhsT=eT[:, 128 * t : 128 * (t + 1)],
                        rhs=qk_sb[:, W * (TPB * sb + t) : W * (TPB * sb + t + 1)],
                        start=True,
                        stop=True,
                    )
                E = pe_.tile([128, TPB * W], bf16)
                nc.scalar.activation(E[:], L[:], mybir.ActivationFunctionType.Exp)
                me = pme.tile([128, TPB * W], bf16)
                nc.vector.tensor_mul(me[:], mk[:], E[:])

                # scatter: numerator + denominator per group slot
                ob = posb.tile([128, 8 * ROWS], bf16)
                for q in range(4):
                    sp = pnm.tile([128, 512], f32, space="PSUM")
                    for r in range(8):
                        t = 8 * q + r
                        j = r % 4
                        k = r // 4
                        nc.tensor.matmul(
                            sp[32 * j : 32 * j + 16, ROWS * k : ROWS * (k + 1)],
                            lhsT=me[:, W * t : W * (t + 1)],
                            rhs=et[:, ROWS * t : ROWS * (t + 1)],
                            start=True,
                            stop=True,
                            tile_position=(0, 32 * j),
                        )
                    nc.vector.tensor_copy(
                        ob[:, 2 * ROWS * q : 2 * ROWS * (q + 1)], sp[:, : 2 * ROWS]
                    )
                # only partitions 32j..32j+15 hold data; ship the valid slices
                for j in range(4):
                    nc.gpsimd.dma_start(
                        out=outp[sb, j], in_=ob[32 * j : 32 * j + 16, :]
                    )

    nc.compile()
    return nc


def _pack_core(counts):
    """Best-fit-decreasing pack of whole groups into tiles (<=128 elems,
    <=W groups per tile).

    Returns tile_of_group, relg_of_group (slot within tile), elem_offset_of_group
    (start position of the group's elements within its tile), n_tiles.
    """
    ng = len(counts)
    order = np.argsort(-counts, kind="stable")
    tile_of_group = np.empty(ng, dtype=np.int64)
    relg_of_group = np.empty(ng, dtype=np.int64)
    off_of_group = np.empty(ng, dtype=np.int64)
    # buckets[c] = list of tile ids with remaining capacity exactly c (and
    # an open group slot)
    buckets = [[] for _ in range(129)]
    cap = []
    slots = []
    for g in order:
        c = int(counts[g])
        # best fit: smallest remaining capacity >= c
        for r in range(c, 129):
            if buckets[r]:
                t = buckets[r].pop()
                break
        else:
            t = len(cap)
            cap.append(128)
            slots.append(0)
        tile_of_group[g] = t
        relg_of_group[g] = slots[t]
        off_of_group[g] = 128 - cap[t]
        cap[t] -= c
        slots[t] += 1
        if slots[t] < W and cap[t] > 0:
            buckets[cap[t]].append(t)
    return tile_of_group, relg_of_group, off_of_group, len(cap)


def _host_prep(embeddings, seg_ids, Wq, bq, Wk, bk, ncores=NCORES, num_groups=G):
    emb = np.ascontiguousarray(embeddings, dtype=np.float32)
    seg = np.ascontiguousarray(seg_ids, dtype=np.int64)
    n = len(seg)

    counts = np.bincount(seg, minlength=num_groups)
    assert counts.max() <= 128, "group larger than one tile"
    cum = np.cumsum(counts)
    starts = cum - counts

    ARm = (Wq.T @ Wk).astype(np.float32)
    uvec = (bq @ Wk).astype(np.float32)

    # split groups across cores at ~equal element counts
    bounds = [0]
    for c in range(1, ncores):
        gidx = int(np.searchsorted(cum, n * c // ncores))
        bounds.append(min(max(gidx, bounds[-1]), num_groups))
    bounds.append(num_groups)

    packs = []
    NT_max = 0
    for c in range(ncores):
        g0, g1 = bounds[c], bounds[c + 1]
        tog, rog, oog, ntile = _pack_core(counts[g0:g1])
        packs.append((g0, g1, tog, rog, oog))
        NT_max = max(NT_max, ntile)
    NSB = (NT_max + TPB - 1) // TPB
    NT = NSB * TPB

    emb_bf = emb.astype(BF16)
    in_maps = []
    decs = []
    for c in range(ncores):
        g0, g1, tog, rog, oog = packs[c]
        e0, e1 = int(starts[g0]), int(cum[g1 - 1])
        segc = seg[e0:e1] - g0
        # element placement
        T_e = tog[segc]                       # tile of each element
        pos_e = (np.arange(e0, e1) - starts[seg[e0:e1]]) + oog[segc]
        sb_e = T_e // TPB
        t_e = T_e % TPB

        nat = np.zeros((NSB, TPB, 128, 128), dtype=BF16)   # [sb, t, i, c]
        nat[sb_e, t_e, pos_e] = emb_bf[e0:e1]

        embt1 = np.empty((NSB, 128, TPB, ROWS), dtype=BF16)
        embt1[:, :, :, 128] = BF16(1.0)
        embt1[:, :, :, :128] = nat.transpose(0, 2, 1, 3)
        embt1 = embt1.reshape(NSB, 128, TPB * ROWS)

        embTt = np.ascontiguousarray(
            nat.transpose(0, 3, 1, 2).reshape(NSB, 128, TPB * 128)
        )
        del nat

        mask = np.zeros((NSB, 128, TPB, W), dtype=ml_dtypes.float8_e4m3)
        mask[sb_e, pos_e, t_e, rog[segc]] = 1.0
        mask = mask.reshape(NSB, 128, TPB * W)

        # host qk table: qk_g = embL_g @ ARm + u
        last_idx = cum[g0:g1] - 1
        qk = emb[last_idx] @ ARm + uvec                     # [ng, 128] f32
        qkT = np.zeros((128, NT * W), dtype=BF16)
        gslot = tog * W + rog                               # [ng]
        qkT[:, gslot] = qk.T.astype(BF16)

        in_maps.append(
            dict(
                embt1=np.ascontiguousarray(embt1),
                embTt=embTt,
                maskh=np.ascontiguousarray(mask),
                qkth=np.ascontiguousarray(qkT),
            )
        )
        # decode indices: group -> outp[sb, j, relg, 258*q + 129*k + c]
        t_in_sb = tog % TPB
        q_g = t_in_sb // 8
        rem = t_in_sb % 8
        j_g = rem % 4
        col_g = 2 * ROWS * q_g + ROWS * (rem // 4)
        decs.append((g0, g1, tog // TPB, j_g, rog, col_g))
    return NSB, in_maps, decs


def kernel(embeddings, seg_ids, Wq, bq, Wk, bk):
    global LAST_EXEC_NS, LAST_RESULTS
    Wq = np.asarray(Wq, dtype=np.float32)
    bq = np.asarray(bq, dtype=np.float32)
    Wk = np.asarray(Wk, dtype=np.float32)
    bk = np.asarray(bk, dtype=np.float32)
    embeddings = np.asarray(embeddings)
    seg_ids = np.asarray(seg_ids)

    NSB, in_maps, decs = _host_prep(embeddings, seg_ids, Wq, bq, Wk, bk)

    if NSB not in _cache:
        _cache[NSB] = _build_program(NSB)
    nc = _cache[NSB]

    trace = bool(int(os.environ.get("BASS_KERNEL_TRACE", "0")))
    res = run_bass_kernel_spmd(nc, in_maps, core_ids=list(range(NCORES)), trace=trace)
    LAST_RESULTS = res
    LAST_EXEC_NS = res.exec_time_ns

    num = np.empty((G, D), dtype=np.float32)
    den = np.empty((G,), dtype=np.float32)
    for c in range(NCORES):
        g0, g1, sb_g, j_g, row_g, col_g = decs[c]
        o = res.results[c]["outp"].astype(np.float32)
        num[g0:g1] = o[sb_g[:, None], j_g[:, None], row_g[:, None], col_g[:, None] + np.arange(128)]
        den[g0:g1] = o[sb_g, j_g, row_g, col_g + 128]
    out = (num / den[:, None]) @ Wk.T + bk
    return out.astype(np.float32)
